# revision 10
# baseline (speedup 1.0000x reference)
"""FBPINN forward kernel for Trainium2 (8 NeuronCores), MoE-routing style.

Strategy
--------
The reference evaluates all S=64 subdomain MLPs densely on all N=131072
points, then combines with a sigmoid-product window w_s(x) normalized over
S.  The window decays like exp(-s_x * d) with s_x ~ 4266 beyond each
subdomain's core cell, so each point has non-negligible w for at most 2
subdomains.  We route points to subdomains on the host (exact interval
test: every dropped (s, point) pair has window sigmoid args <= -TAU, i.e.
w < 3.4e-4 of the normalized sum), pad each subdomain's point list to a
common PAD, and evaluate the tiny MLPs on device, expert-parallel: 8
subdomains per core, packed 4-at-a-time into block-diagonal 128-wide
matmuls.

Device pipeline (per core, 2 groups of 4 subdomains):
  x -> [block-diag in-proj; input normalization and bias folded into the
  weights via a ones row] -> tanh -> 2x [block-diag 32x32 hidden +
  per-partition bias] -> tanh -> block-diag out-proj.
Matmuls run in float32r mode (1 PE cycle/row at >=256 moving cols vs 4
for fp32; measured ~1e-3 rel err, gate is 2e-2), so the Activation
engine's three tanh passes (0.833 ns/col each) are the bottleneck.  The
kernel is organized around keeping ACT busy back-to-back:
  - weights ship as a compact 33KB staging blob + 4.6KB in-proj blob per
    group (not the 200KB dense block-diag) and are expanded on device by
    DVE copies into a zeroed block-diag tile, so input DMA never gates
    compute;
  - x ships per column-tile so the first matmul starts ~2.5us in;
  - tiles are emitted in PAIRS with double-buffered PSUM so the tanh of
    tile B hides the matmul+semaphore latency between tile A's layers.
Host does: routing, window weights, scatter-add normalization, boundary
condition.  Cross-subdomain reduction happens in the host scatter-add,
so no collectives are needed.
"""

import numpy as np
from contextlib import ExitStack

S = 64
N_DIM = 2
H = 32
SCALE, SHIFT = 1.0, 0.0
NCORES = 8
SUB_PER_CORE = S // NCORES      # 8
G = 2                           # groups of 4 subdomains per core
TAU = 8.0                       # dropped window weight <= sigmoid(-TAU) ~ 3.4e-4
T = 512                         # device column tile

_BUILD_CACHE = {}


def _tile_sizes(pad):
    """Column tiles of <=512, all >=256 so float32r runs at 1 cycle/row."""
    full, rem = divmod(pad, T)
    if rem == 0:
        return [T] * full
    if rem >= 256:
        return [T] * full + [rem]
    # rem == 128: split the last 640 into 384+256
    return [T] * (full - 1) + [384, 256]


def _build_bass(pad):
    import concourse.bass as bass
    import concourse.tile as tile
    from concourse import bacc, mybir

    f32 = mybir.dt.float32
    f32r = mybir.dt.float32r
    nc = bacc.Bacc("TRN2", target_bir_lowering=False, debug=False,
                   num_devices=NCORES)
    xb = nc.dram_tensor("xb", [G, 9, pad], f32r, kind="ExternalInput").ap()
    wic = nc.dram_tensor("wic", [G, 9, 128], f32r, kind="ExternalInput").ap()
    ws = nc.dram_tensor("ws", [G, 128, 67], f32r, kind="ExternalInput").ap()
    o = nc.dram_tensor("o", [G, 4, pad], f32, kind="ExternalOutput").ap()

    tanh = mybir.ActivationFunctionType.Tanh
    sizes = _tile_sizes(pad)
    offs = [sum(sizes[:i]) for i in range(len(sizes))]

    with tile.TileContext(nc) as tc, ExitStack() as ctx:
        consts = ctx.enter_context(tc.tile_pool(name="consts", bufs=1))
        hpool = ctx.enter_context(tc.tile_pool(name="hs", bufs=2))
        opool = ctx.enter_context(tc.tile_pool(name="os", bufs=1))
        psum = ctx.enter_context(tc.tile_pool(name="ps", bufs=2, space="PSUM"))

        # --- warm-up first: tanh table load + a fat fp32 burst that keeps the
        # PE busy for one full HAM window so the 2.4GHz clock is up before the
        # first real tile, all while the input DMAs are in flight.
        warm = hpool.tile([128, 512], f32, tag="warm", bufs=1, name="warm")
        nc.vector.memset(warm[:], 0.0)
        warm_h = consts.tile([1, 1], f32, tag="wh", name="warmh")
        nc.scalar.activation(warm_h[:], warm[0:1, 0:1], tanh)

        # --- weight staging + block-diag expansion, and chunked xb DMAs.
        # wb layout (as in the dense baseline): wi [0:9,0:128], wh0
        # [:,128:256], wh1 [:,256:384], wo [:,384:388], bh0/bh1 [:,388:390].
        xb_t, wb_t, ws_t = {}, {}, {}
        for g in range(G):
            wb_t[g] = consts.tile([128, 390], f32r, tag=f"wb{g}", name=f"wbt{g}")
            ws_t[g] = consts.tile([128, 67], f32r, tag=f"ws{g}", name=f"wst{g}")
            xb_t[g] = consts.tile([9, pad], f32r, tag=f"xb{g}", name=f"xbt{g}")
            nc.vector.memset(wb_t[g][:, 128:390].bitcast(f32), 0.0)
        # weight blobs ride the gpsimd software DGE (25ns issue) so the SP
        # hwdge queue serialization (~730ns per dma_start) only carries xb.
        nc.gpsimd.dma_start(out=ws_t[0][:], in_=ws[0])
        nc.gpsimd.dma_start(out=wb_t[0][0:9, 0:128], in_=wic[0])
        nc.gpsimd.dma_start(out=ws_t[1][:], in_=ws[1])
        nc.gpsimd.dma_start(out=wb_t[1][0:9, 0:128], in_=wic[1])
        # xb chunks on SP, first-needed first; chunk 0 covers pair 0 exactly.
        xchunks = {0: [1024, 704, 704], 1: [1216, 1216]}
        for g, csz_list in ((0, xchunks[0]), (1, xchunks[1])):
            off = 0
            for csz in csz_list:
                nc.sync.dma_start(out=xb_t[g][:, off:off + csz],
                                  in_=xb[g][:, off:off + csz])
                off += csz

        # Expand staging -> block-diag on DVE (same partition range on both
        # sides keeps every copy affine).
        for g in range(G):
            for j in range(4):
                r = slice(32 * j, 32 * j + 32)
                nc.vector.tensor_copy(wb_t[g][r, 128 + 32 * j:160 + 32 * j],
                                      ws_t[g][r, 0:32])
                nc.vector.tensor_copy(wb_t[g][r, 256 + 32 * j:288 + 32 * j],
                                      ws_t[g][r, 32:64])
                nc.vector.tensor_copy(wb_t[g][r, 384 + j:385 + j],
                                      ws_t[g][r, 64:65])
            nc.vector.tensor_copy(wb_t[g][:, 388:390], ws_t[g][:, 65:67])

        wi_t = {g: wb_t[g][0:9, 0:128] for g in range(G)}
        wh_t = {(g, l): wb_t[g][:, 128 * (l + 1):128 * (l + 2)]
                for g in range(G) for l in range(2)}
        wo_t = {g: wb_t[g][:, 384:388] for g in range(G)}
        bh_t = {(g, l): wb_t[g][:, 388 + l:389 + l].bitcast(f32)
                for g in range(G) for l in range(2)}

        # --- main pipeline: tiles across both groups, processed in pairs.
        tiles = [(g, offs[i], sizes[i]) for g in range(G)
                 for i in range(len(sizes))]
        pairs = [tiles[i:i + 2] for i in range(0, len(tiles), 2)]

        o_sb = {g: opool.tile([4, pad], f32, tag=f"o{g}", name=f"osb{g}")
                for g in range(G)}

        def make_p1(pi):
            return psum.tile([128, 1024], f32, tag="p1", bufs=2,
                             padded_shape=[128, 1024], name=f"p1_{pi}")

        def emit_mm1(pair, p1):
            # each tile gets its own PSUM bank: matmul output must not cross
            # the 512-col bank boundary
            cols = {t: 512 * i for i, t in enumerate(pair)}
            for t in pair:
                g, off, tsz = t
                c0 = cols[t]
                nc.tensor.matmul(p1[:, c0:c0 + tsz], wi_t[g],
                                 xb_t[g][:, off:off + tsz],
                                 start=True, stop=True)
            return cols

        def junk_mm(p1, ncols):
            # fp32 throwaway into dead rows of an already-consumed p1: keeps
            # the PE HAM activity window busy so the clock stays at 2.4GHz
            nc.tensor.matmul(p1[32:36, 0:ncols], warm[:, 0:4], warm[:, 0:ncols],
                             start=True, stop=True, skip_group_check=True)

        def emit_mm4s(pend):
            # out-proj into dead rows 0:4 of the pair's own (consumed) p3
            # tiles, then DVE copy + one output DMA per contiguous run.
            pair, p3s, h3s = pend
            for t in pair:
                g, off, tsz = t
                nc.tensor.matmul(p3s[t][0:4, :], wo_t[g], h3s[t][:],
                                 start=True, stop=True, skip_group_check=True)
            for t in pair:
                g, off, tsz = t
                nc.vector.tensor_copy(o_sb[g][:, off:off + tsz],
                                      p3s[t][0:4, :])
            merged = (len(pair) == 2 and pair[0][0] == pair[1][0]
                      and pair[0][1] + pair[0][2] == pair[1][1])
            if merged:
                g, off, _ = pair[0]
                tot = pair[0][2] + pair[1][2]
                nc.sync.dma_start(out=o[g][:, off:off + tot],
                                  in_=o_sb[g][:, off:off + tot])
            else:
                for t in pair:
                    g, off, tsz = t
                    nc.sync.dma_start(out=o[g][:, off:off + tsz],
                                      in_=o_sb[g][:, off:off + tsz])

        p1_cur = make_p1(0)
        for i in range(2):
            junk_mm(p1_cur, 512)        # pre-warm burst (~3.4us fp32 @ cold)
        cols_cur = emit_mm1(pairs[0], p1_cur)

        pend = None
        for pi, pair in enumerate(pairs):
            p1, cols = p1_cur, cols_cur
            h1s, p2s, h2s, p3s, h3s = {}, {}, {}, {}, {}
            for t in pair:
                g, off, tsz = t
                c0 = cols[t]
                h1s[t] = hpool.tile([128, tsz], f32r, tag="h1", bufs=2,
                                    padded_shape=[128, T], name=f"h1_{g}_{off}")
                nc.scalar.activation(h1s[t][:], p1[:, c0:c0 + tsz], tanh)
            for t in pair:
                g, off, tsz = t
                p2s[t] = psum.tile([128, tsz], f32, tag="p2",
                                   padded_shape=[128, T], name=f"p2_{g}_{off}")
                nc.tensor.matmul(p2s[t][:], wh_t[g, 0], h1s[t][:],
                                 start=True, stop=True)
            if pend is not None:
                emit_mm4s(pend)
                pend = None
            for t in pair:
                g, off, tsz = t
                h2s[t] = hpool.tile([128, tsz], f32r, tag="h2", bufs=2,
                                    padded_shape=[128, T], name=f"h2_{g}_{off}")
                nc.scalar.activation(h2s[t][:], p2s[t][:], tanh, bias=bh_t[g, 0])
            for t in pair:
                g, off, tsz = t
                p3s[t] = psum.tile([128, tsz], f32, tag="p3",
                                   padded_shape=[128, T], name=f"p3_{g}_{off}")
                nc.tensor.matmul(p3s[t][:], wh_t[g, 1], h2s[t][:],
                                 start=True, stop=True)
            if pi + 1 < len(pairs):
                p1_cur = make_p1(pi + 1)
                cols_cur = emit_mm1(pairs[pi + 1], p1_cur)
            junk_mm(p1, 256)            # keep the HAM window busy
            for t in pair:
                g, off, tsz = t
                h3s[t] = hpool.tile([128, tsz], f32r, tag="h3", bufs=4,
                                    padded_shape=[128, T], name=f"h3_{g}_{off}")
                nc.scalar.activation(h3s[t][:], p3s[t][:], tanh, bias=bh_t[g, 1])
            pend = (pair, p3s, h3s)
        emit_mm4s(pend)
    nc.compile()
    return nc


def _route(x, lo_core, hi_core, swin):
    """Per-subdomain point lists: s covers p iff all window sigmoid args >= -TAU."""
    n = x.shape[0]
    pts = []
    for si in range(S):
        m = np.ones(n, dtype=bool)
        for d in range(N_DIM):
            sd = swin[si, d]
            lo, hi = lo_core[si, d], hi_core[si, d]
            if sd >= 0:
                m &= (x[:, d] >= lo - TAU / max(sd, 1e-30)) \
                    & (x[:, d] <= hi + TAU / max(sd, 1e-30))
            else:  # pathological geometry; sigmoids flip direction
                m &= (x[:, d] <= lo + TAU / max(-sd, 1e-30)) \
                    & (x[:, d] >= hi - TAU / max(-sd, 1e-30))
        pts.append(np.nonzero(m)[0])
    return pts


def _pack(x, args64, pts, pad, Wn, bn):
    """Build the per-core device input tensors (compact weight layout)."""
    W_h1 = args64["W_h1"].astype(np.float32)
    W_h2 = args64["W_h2"].astype(np.float32)
    W_out = args64["W_out"].astype(np.float32)
    b_h1 = args64["b_h1"].astype(np.float32)
    b_h2 = args64["b_h2"].astype(np.float32)
    in_maps = []
    for c in range(NCORES):
        xb = np.zeros((G, 9, pad), np.float32)
        wic = np.zeros((G, 9, 128), np.float32)
        wsv = np.zeros((G, 128, 67), np.float32)
        for g in range(G):
            xb[g, 0, :] = 1.0
            for j in range(4):
                s_ = c * SUB_PER_CORE + g * 4 + j
                idx = pts[s_]
                cnt = len(idx)
                xs = x[idx]
                xb[g, 1 + 2 * j, :cnt] = xs[:, 0]
                xb[g, 2 + 2 * j, :cnt] = xs[:, 1]
                r = slice(32 * j, 32 * j + 32)
                for d in range(N_DIM):
                    wic[g, 1 + 2 * j + d, r] = Wn[s_, :, d]
                wic[g, 0, r] = bn[s_]
                wsv[g, r, 0:32] = W_h1[s_].T
                wsv[g, r, 32:64] = W_h2[s_].T
                wsv[g, r, 64] = W_out[s_, 0]
                wsv[g, r, 65] = b_h1[s_]
                wsv[g, r, 66] = b_h2[s_]
        in_maps.append({"xb": xb, "wic": wic, "ws": wsv})
    return in_maps


def _host_reference(x, lo_core, hi_core, lo_ext, hi_ext,
                    W_in, b_in, W_h1, b_h1, W_h2, b_h2, W_out, b_out):
    """Dense fallback (numpy, chunked) for inputs without FBPINN locality."""
    center = (lo_ext + hi_ext) * 0.5
    half_w = (hi_ext - lo_ext) * 0.5
    overlap = np.maximum(hi_ext - hi_core, lo_core - lo_ext)
    width = hi_ext - lo_ext
    s = 4.0 / (2.0 * overlap * width + 1e-8)
    sigm = lambda v: 1.0 / (1.0 + np.exp(-v))
    outs = []
    for i in range(0, x.shape[0], 8192):
        xc = x[i:i + 8192].astype(np.float64)
        xn = (xc[None] - center[:, None]) / half_w[:, None]
        hh = np.tanh(np.einsum("snd,shd->snh", xn, W_in) + b_in[:, None])
        hh = np.tanh(np.einsum("snh,skh->snk", hh, W_h1) + b_h1[:, None])
        hh = np.tanh(np.einsum("snh,skh->snk", hh, W_h2) + b_h2[:, None])
        out = np.einsum("snh,soh->sno", hh, W_out) + b_out[:, None]
        out = out * SCALE + SHIFT
        left = sigm(s[:, None] * (xc[None] - lo_core[:, None]))
        right = sigm(s[:, None] * (hi_core[:, None] - xc[None]))
        w = np.prod(left * right, axis=-1, keepdims=True)
        w = w / (np.sum(w, axis=0, keepdims=True) + 1e-8)
        u = np.sum(out * w, axis=0)
        gg = -np.sin(np.pi * xc[:, 1])[:, None]
        fac = (np.tanh(xc[:, 1] + 1) * np.tanh(xc[:, 1] - 1)
               * np.tanh(xc[:, 0]))[:, None]
        outs.append((gg + fac * u).astype(np.float32))
    return np.concatenate(outs, axis=0)


def _prepare(x, args64):
    """Routing + weight folding. Returns (pts, pad, swin, Wn, bn) or None
    if the inputs lack FBPINN locality (caller should fall back to dense)."""
    lo_core64, hi_core64 = args64["lo_core"], args64["hi_core"]
    lo_ext64, hi_ext64 = args64["lo_ext"], args64["hi_ext"]
    n = x.shape[0]
    center = (lo_ext64 + hi_ext64) * 0.5
    half_w = (hi_ext64 - lo_ext64) * 0.5
    overlap = np.maximum(hi_ext64 - hi_core64, lo_core64 - lo_ext64)
    width = hi_ext64 - lo_ext64
    swin = 4.0 / (2.0 * overlap * width + 1e-8)

    pts = _route(x, lo_core64, hi_core64, swin)
    counts = np.array([len(p) for p in pts])
    if counts.sum() > 4 * n or counts.max() > max(4 * n // S, 8192):
        return None
    pad = int(max(256, -(-counts.max() // 128) * 128))

    W_in64 = args64["W_in"]                      # (S,H,D)
    Wn = W_in64 / half_w[:, None, :]             # (S,H,D)
    bn = args64["b_in"] - np.einsum("shd,sd->sh", W_in64, center / half_w)
    return pts, pad, swin, Wn, bn


def _epilogue(x, args64, pts, swin, o_by_sub):
    """Window weights + normalized scatter-add + boundary condition.
    o_by_sub: callable s -> raw device MLP outputs for subdomain s's slots."""
    n = x.shape[0]
    lo_core64, hi_core64 = args64["lo_core"], args64["hi_core"]
    b_out64 = args64["b_out"]
    numer = np.zeros(n, np.float64)
    denom = np.zeros(n, np.float64)
    sigm = lambda v: 1.0 / (1.0 + np.exp(-v))
    for s_ in range(S):
        idx = pts[s_]
        cnt = len(idx)
        if cnt == 0:
            continue
        xs = x[idx].astype(np.float64)
        arg_l = swin[s_] * (xs - lo_core64[s_])
        arg_r = swin[s_] * (hi_core64[s_] - xs)
        w = np.prod(sigm(arg_l) * sigm(arg_r), axis=-1)
        out_s = (o_by_sub(s_)[:cnt].astype(np.float64)
                 + b_out64[s_, 0]) * SCALE + SHIFT
        np.add.at(numer, idx, out_s * w)
        np.add.at(denom, idx, w)
    u = numer / (denom + 1e-8)
    x64 = x.astype(np.float64)
    gg = -np.sin(np.pi * x64[:, 1])
    fac = np.tanh(x64[:, 1] + 1.0) * np.tanh(x64[:, 1] - 1.0) * np.tanh(x64[:, 0])
    return (gg + fac * u)[:, None].astype(np.float32)


def kernel(x, lo_core, hi_core, lo_ext, hi_ext,
           W_in, b_in, W_h1, b_h1, W_h2, b_h2, W_out, b_out,
           _profile=False):
    x = np.asarray(x, np.float32)
    args64 = {k: np.asarray(v, np.float64) for k, v in dict(
        lo_core=lo_core, hi_core=hi_core, lo_ext=lo_ext, hi_ext=hi_ext,
        W_in=W_in, b_in=b_in, W_h1=W_h1, b_h1=b_h1, W_h2=W_h2, b_h2=b_h2,
        W_out=W_out, b_out=b_out).items()}

    prep = _prepare(x, args64)
    if prep is None:
        return _host_reference(x, **args64)
    pts, pad, swin, Wn, bn = prep

    in_maps = _pack(x, args64, pts, pad, Wn, bn)

    from concourse.bass_utils import run_bass_kernel_spmd
    if pad not in _BUILD_CACHE:
        _BUILD_CACHE[pad] = _build_bass(pad)
    nc = _BUILD_CACHE[pad]
    res = run_bass_kernel_spmd(nc, in_maps, list(range(NCORES)),
                               trace=bool(_profile))

    def o_by_sub(s_):
        c, rem = divmod(s_, SUB_PER_CORE)
        g, j = divmod(rem, 4)
        return res.results[c]["o"][g, j]

    final = _epilogue(x, args64, pts, swin, o_by_sub)
    if _profile:
        return final, res
    return final


# revision 11
# speedup vs baseline: 1.1090x; 1.1090x over previous
"""FBPINN forward kernel for Trainium2 (8 NeuronCores), MoE-routing style.

Strategy
--------
The reference evaluates all S=64 subdomain MLPs densely on all N=131072
points, then combines with a sigmoid-product window w_s(x) normalized over
S.  The window decays like exp(-s_x * d) with s_x ~ 4266 beyond each
subdomain's core cell, so each point has non-negligible w for at most 2
subdomains.  We route points to subdomains on the host (exact interval
test: every dropped (s, point) pair has window sigmoid args <= -TAU, i.e.
w < 3.4e-4 of the normalized sum), pad each subdomain's point list to a
common PAD, and evaluate the tiny MLPs on device, expert-parallel: 8
subdomains per core, packed 4-at-a-time into block-diagonal 128-wide
matmuls.

Device pipeline (per core, 2 groups of 4 subdomains):
  x -> [block-diag in-proj; input normalization and bias folded into the
  weights via a ones row] -> tanh -> 2x [block-diag 32x32 hidden +
  per-partition bias] -> tanh -> block-diag out-proj.
Matmuls run in float32r mode (1 PE cycle/row at >=256 moving cols vs 4
for fp32; measured ~1e-3 rel err, gate is 2e-2), so the Activation
engine's three tanh passes (0.833 ns/col each) are the bottleneck.  The
kernel is organized around keeping ACT busy back-to-back:
  - weights ship as a compact 33KB staging blob + 4.6KB in-proj blob per
    group (not the 200KB dense block-diag) and are expanded on device by
    DVE copies into a zeroed block-diag tile, so input DMA never gates
    compute;
  - x ships per column-tile so the first matmul starts ~2.5us in;
  - tiles are emitted in PAIRS with double-buffered PSUM so the tanh of
    tile B hides the matmul+semaphore latency between tile A's layers.
Host does: routing, window weights, scatter-add normalization, boundary
condition.  Cross-subdomain reduction happens in the host scatter-add,
so no collectives are needed.
"""

import numpy as np
from contextlib import ExitStack

S = 64
N_DIM = 2
H = 32
SCALE, SHIFT = 1.0, 0.0
NCORES = 8
SUB_PER_CORE = S // NCORES      # 8
G = 2                           # groups of 4 subdomains per core
TAU = 8.0                       # dropped window weight <= sigmoid(-TAU) ~ 3.4e-4
T = 512                         # device column tile

_BUILD_CACHE = {}


def _tile_sizes(pad):
    """Column tiles of <=512, all >=256 so float32r runs at 1 cycle/row."""
    full, rem = divmod(pad, T)
    if rem == 0:
        return [T] * full
    if rem >= 256:
        return [T] * full + [rem]
    # rem == 128: split the last 640 into 384+256
    return [T] * (full - 1) + [384, 256]


def _build_bass(pad):
    import concourse.bass as bass
    import concourse.tile as tile
    from concourse import bacc, mybir

    f32 = mybir.dt.float32
    f32r = mybir.dt.float32r
    nc = bacc.Bacc("TRN2", target_bir_lowering=False, debug=False,
                   num_devices=NCORES)
    xb = nc.dram_tensor("xb", [G, 9, pad], f32r, kind="ExternalInput").ap()
    wic = nc.dram_tensor("wic", [G, 9, 128], f32r, kind="ExternalInput").ap()
    ws = nc.dram_tensor("ws", [G, 128, 67], f32r, kind="ExternalInput").ap()
    o = nc.dram_tensor("o", [G, 4, pad], f32, kind="ExternalOutput").ap()

    tanh = mybir.ActivationFunctionType.Tanh
    sizes = _tile_sizes(pad)
    offs = [sum(sizes[:i]) for i in range(len(sizes))]

    with tile.TileContext(nc) as tc, ExitStack() as ctx:
        consts = ctx.enter_context(tc.tile_pool(name="consts", bufs=1))
        hpool = ctx.enter_context(tc.tile_pool(name="hs", bufs=2))
        opool = ctx.enter_context(tc.tile_pool(name="os", bufs=1))
        psum = ctx.enter_context(tc.tile_pool(name="ps", bufs=2, space="PSUM"))

        # --- warm-up first: tanh table load + a fat fp32 burst that keeps the
        # PE busy for one full HAM window so the 2.4GHz clock is up before the
        # first real tile, all while the input DMAs are in flight.
        warm = hpool.tile([128, 512], f32, tag="warm", bufs=1, name="warm")
        nc.vector.memset(warm[:], 0.0)
        warm_h = consts.tile([1, 1], f32, tag="wh", name="warmh")
        nc.scalar.activation(warm_h[:], warm[0:1, 0:1], tanh)

        # --- weight staging + block-diag expansion, and chunked xb DMAs.
        # wb layout (as in the dense baseline): wi [0:9,0:128], wh0
        # [:,128:256], wh1 [:,256:384], wo [:,384:388], bh0/bh1 [:,388:390].
        xb_t, wb_t, ws_t = {}, {}, {}
        for g in range(G):
            wb_t[g] = consts.tile([128, 390], f32r, tag=f"wb{g}", name=f"wbt{g}")
            ws_t[g] = consts.tile([128, 67], f32r, tag=f"ws{g}", name=f"wst{g}")
            xb_t[g] = consts.tile([9, pad], f32r, tag=f"xb{g}", name=f"xbt{g}")
            nc.vector.memset(wb_t[g][:, 128:390].bitcast(f32), 0.0)
        # weight blobs ride the gpsimd software DGE (25ns issue) so the SP
        # hwdge queue serialization (~730ns per dma_start) only carries xb.
        nc.gpsimd.dma_start(out=ws_t[0][:], in_=ws[0])
        nc.gpsimd.dma_start(out=wb_t[0][0:9, 0:128], in_=wic[0])
        nc.gpsimd.dma_start(out=ws_t[1][:], in_=ws[1])
        nc.gpsimd.dma_start(out=wb_t[1][0:9, 0:128], in_=wic[1])
        # xb chunks on SP, first-needed first; chunk 0 covers pair 0 exactly.
        c0 = min(512, pad)
        r0 = pad - c0
        xchunks = {0: [c0] + ([(r0 + 1) // 2, r0 // 2] if r0 else []),
                   1: [(pad + 1) // 2, pad // 2]}
        for g, csz_list in ((0, xchunks[0]), (1, xchunks[1])):
            off = 0
            for csz in csz_list:
                nc.sync.dma_start(out=xb_t[g][:, off:off + csz],
                                  in_=xb[g][:, off:off + csz])
                off += csz

        # Expand staging -> block-diag on DVE (same partition range on both
        # sides keeps every copy affine).
        for g in range(G):
            for j in range(4):
                r = slice(32 * j, 32 * j + 32)
                nc.vector.tensor_copy(wb_t[g][r, 128 + 32 * j:160 + 32 * j],
                                      ws_t[g][r, 0:32])
                nc.vector.tensor_copy(wb_t[g][r, 256 + 32 * j:288 + 32 * j],
                                      ws_t[g][r, 32:64])
                nc.vector.tensor_copy(wb_t[g][r, 384 + j:385 + j],
                                      ws_t[g][r, 64:65])
            nc.vector.tensor_copy(wb_t[g][:, 388:390], ws_t[g][:, 65:67])

        wi_t = {g: wb_t[g][0:9, 0:128] for g in range(G)}
        wh_t = {(g, l): wb_t[g][:, 128 * (l + 1):128 * (l + 2)]
                for g in range(G) for l in range(2)}
        wo_t = {g: wb_t[g][:, 384:388] for g in range(G)}
        bh_t = {(g, l): wb_t[g][:, 388 + l:389 + l].bitcast(f32)
                for g in range(G) for l in range(2)}

        # --- main pipeline: a 3-deep diagonal software pipeline over the 10
        # column tiles.  Diagonal d runs tanh-1 of tile d, tanh-2 of tile
        # d-1, tanh-3 of tile d-2 on ACT while PE fills the matmuls for the
        # adjacent diagonals, so ACT never waits on matmul+semaphore latency
        # even when the PE clock gate is cold.  PSUM: p1 x2 + p2 x3 + p3 x3
        # banks = 8.  The out-proj of tile d-3 lands in dead rows 0:4 of its
        # own already-consumed p3 bank.
        tiles = [(g, offs[i], sizes[i]) for g in range(G)
                 for i in range(len(sizes))]
        n = len(tiles)

        o_sb = {g: opool.tile([4, pad], f32, tag=f"o{g}", name=f"osb{g}")
                for g in range(G)}

        p1t, h1t, p2t, h2t, p3t, h3t = {}, {}, {}, {}, {}, {}

        def ensure_p1(k):
            p1t[k] = psum.tile([128, tiles[k][2]], f32, tag="p1", bufs=2,
                               padded_shape=[128, T], name=f"p1_{k}")

        def mm1(k):
            g, off, tsz = tiles[k]
            nc.tensor.matmul(p1t[k][:], wi_t[g], xb_t[g][:, off:off + tsz],
                             start=True, stop=True)

        ensure_p1(0)
        for i in range(2):
            # pre-warm burst: fat fp32 throwaways into rows that mm1(t0)
            # will overwrite; ~3.5us of sustained PE busy flips the HAM
            # clock gate to 2.4GHz before the first real tile
            nc.tensor.matmul(p1t[0][32:36, 0:512], warm[:, 0:4], warm[:],
                             start=True, stop=True, skip_group_check=True)
        mm1(0)
        ensure_p1(1)
        mm1(1)

        for d in range(n + 3):
            if d < n:
                g, off, tsz = tiles[d]
                h1t[d] = hpool.tile([128, tsz], f32r, tag="h1", bufs=2,
                                    padded_shape=[128, T], name=f"h1_{d}")
                nc.scalar.activation(h1t[d][:], p1t[d][:], tanh)
            if d - 3 >= 0:
                g3, off3, tsz3 = tiles[d - 3]
                nc.tensor.matmul(p3t[d - 3][0:4, :], wo_t[g3], h3t[d - 3][:],
                                 start=True, stop=True, skip_group_check=True)
            if d < n:
                g, off, tsz = tiles[d]
                p2t[d] = psum.tile([128, tsz], f32, tag="p2", bufs=3,
                                   padded_shape=[128, T], name=f"p2_{d}")
                nc.tensor.matmul(p2t[d][:], wh_t[g, 0], h1t[d][:],
                                 start=True, stop=True)
            if d + 2 < n:
                ensure_p1(d + 2)
                mm1(d + 2)
            if 0 <= d - 1 < n:
                g1_, off1, tsz1 = tiles[d - 1]
                h2t[d - 1] = hpool.tile([128, tsz1], f32r, tag="h2", bufs=2,
                                        padded_shape=[128, T],
                                        name=f"h2_{d - 1}")
                nc.scalar.activation(h2t[d - 1][:], p2t[d - 1][:], tanh,
                                     bias=bh_t[g1_, 0])
                p3t[d - 1] = psum.tile([128, tsz1], f32, tag="p3", bufs=3,
                                       padded_shape=[128, T],
                                       name=f"p3_{d - 1}")
                nc.tensor.matmul(p3t[d - 1][:], wh_t[g1_, 1], h2t[d - 1][:],
                                 start=True, stop=True)
            if d - 3 >= 0:
                # tiny fp32 throwaway hides the preceding matmul's pipe-drain
                # and keeps the HAM activity window from going idle
                nc.tensor.matmul(p3t[d - 3][32:36, 0:64], warm[:, 0:4],
                                 warm[:, 0:64], start=True, stop=True,
                                 skip_group_check=True)
            if 0 <= d - 2 < n:
                g2_, off2, tsz2 = tiles[d - 2]
                h3t[d - 2] = hpool.tile([128, tsz2], f32r, tag="h3", bufs=4,
                                        padded_shape=[128, T],
                                        name=f"h3_{d - 2}")
                nc.scalar.activation(h3t[d - 2][:], p3t[d - 2][:], tanh,
                                     bias=bh_t[g2_, 1])
            if d - 3 >= 0:
                g3, off3, tsz3 = tiles[d - 3]
                nc.vector.tensor_copy(o_sb[g3][:, off3:off3 + tsz3],
                                      p3t[d - 3][0:4, :])
                nc.sync.dma_start(out=o[g3][:, off3:off3 + tsz3],
                                  in_=o_sb[g3][:, off3:off3 + tsz3])
    nc.compile()
    return nc


def _route(x, lo_core, hi_core, swin):
    """Per-subdomain point lists: s covers p iff all window sigmoid args >= -TAU."""
    n = x.shape[0]
    pts = []
    for si in range(S):
        m = np.ones(n, dtype=bool)
        for d in range(N_DIM):
            sd = swin[si, d]
            lo, hi = lo_core[si, d], hi_core[si, d]
            if sd >= 0:
                m &= (x[:, d] >= lo - TAU / max(sd, 1e-30)) \
                    & (x[:, d] <= hi + TAU / max(sd, 1e-30))
            else:  # pathological geometry; sigmoids flip direction
                m &= (x[:, d] <= lo + TAU / max(-sd, 1e-30)) \
                    & (x[:, d] >= hi - TAU / max(-sd, 1e-30))
        pts.append(np.nonzero(m)[0])
    return pts


def _pack(x, args64, pts, pad, Wn, bn):
    """Build the per-core device input tensors (compact weight layout)."""
    W_h1 = args64["W_h1"].astype(np.float32)
    W_h2 = args64["W_h2"].astype(np.float32)
    W_out = args64["W_out"].astype(np.float32)
    b_h1 = args64["b_h1"].astype(np.float32)
    b_h2 = args64["b_h2"].astype(np.float32)
    in_maps = []
    for c in range(NCORES):
        xb = np.zeros((G, 9, pad), np.float32)
        wic = np.zeros((G, 9, 128), np.float32)
        wsv = np.zeros((G, 128, 67), np.float32)
        for g in range(G):
            xb[g, 0, :] = 1.0
            for j in range(4):
                s_ = c * SUB_PER_CORE + g * 4 + j
                idx = pts[s_]
                cnt = len(idx)
                xs = x[idx]
                xb[g, 1 + 2 * j, :cnt] = xs[:, 0]
                xb[g, 2 + 2 * j, :cnt] = xs[:, 1]
                r = slice(32 * j, 32 * j + 32)
                for d in range(N_DIM):
                    wic[g, 1 + 2 * j + d, r] = Wn[s_, :, d]
                wic[g, 0, r] = bn[s_]
                wsv[g, r, 0:32] = W_h1[s_].T
                wsv[g, r, 32:64] = W_h2[s_].T
                wsv[g, r, 64] = W_out[s_, 0]
                wsv[g, r, 65] = b_h1[s_]
                wsv[g, r, 66] = b_h2[s_]
        in_maps.append({"xb": xb, "wic": wic, "ws": wsv})
    return in_maps


def _host_reference(x, lo_core, hi_core, lo_ext, hi_ext,
                    W_in, b_in, W_h1, b_h1, W_h2, b_h2, W_out, b_out):
    """Dense fallback (numpy, chunked) for inputs without FBPINN locality."""
    center = (lo_ext + hi_ext) * 0.5
    half_w = (hi_ext - lo_ext) * 0.5
    overlap = np.maximum(hi_ext - hi_core, lo_core - lo_ext)
    width = hi_ext - lo_ext
    s = 4.0 / (2.0 * overlap * width + 1e-8)
    sigm = lambda v: 1.0 / (1.0 + np.exp(-v))
    outs = []
    for i in range(0, x.shape[0], 8192):
        xc = x[i:i + 8192].astype(np.float64)
        xn = (xc[None] - center[:, None]) / half_w[:, None]
        hh = np.tanh(np.einsum("snd,shd->snh", xn, W_in) + b_in[:, None])
        hh = np.tanh(np.einsum("snh,skh->snk", hh, W_h1) + b_h1[:, None])
        hh = np.tanh(np.einsum("snh,skh->snk", hh, W_h2) + b_h2[:, None])
        out = np.einsum("snh,soh->sno", hh, W_out) + b_out[:, None]
        out = out * SCALE + SHIFT
        left = sigm(s[:, None] * (xc[None] - lo_core[:, None]))
        right = sigm(s[:, None] * (hi_core[:, None] - xc[None]))
        w = np.prod(left * right, axis=-1, keepdims=True)
        w = w / (np.sum(w, axis=0, keepdims=True) + 1e-8)
        u = np.sum(out * w, axis=0)
        gg = -np.sin(np.pi * xc[:, 1])[:, None]
        fac = (np.tanh(xc[:, 1] + 1) * np.tanh(xc[:, 1] - 1)
               * np.tanh(xc[:, 0]))[:, None]
        outs.append((gg + fac * u).astype(np.float32))
    return np.concatenate(outs, axis=0)


def _prepare(x, args64):
    """Routing + weight folding. Returns (pts, pad, swin, Wn, bn) or None
    if the inputs lack FBPINN locality (caller should fall back to dense)."""
    lo_core64, hi_core64 = args64["lo_core"], args64["hi_core"]
    lo_ext64, hi_ext64 = args64["lo_ext"], args64["hi_ext"]
    n = x.shape[0]
    center = (lo_ext64 + hi_ext64) * 0.5
    half_w = (hi_ext64 - lo_ext64) * 0.5
    overlap = np.maximum(hi_ext64 - hi_core64, lo_core64 - lo_ext64)
    width = hi_ext64 - lo_ext64
    swin = 4.0 / (2.0 * overlap * width + 1e-8)

    pts = _route(x, lo_core64, hi_core64, swin)
    counts = np.array([len(p) for p in pts])
    if counts.sum() > 4 * n or counts.max() > max(4 * n // S, 8192):
        return None
    pad = int(max(256, -(-counts.max() // 128) * 128))

    W_in64 = args64["W_in"]                      # (S,H,D)
    Wn = W_in64 / half_w[:, None, :]             # (S,H,D)
    bn = args64["b_in"] - np.einsum("shd,sd->sh", W_in64, center / half_w)
    return pts, pad, swin, Wn, bn


def _epilogue(x, args64, pts, swin, o_by_sub):
    """Window weights + normalized scatter-add + boundary condition.
    o_by_sub: callable s -> raw device MLP outputs for subdomain s's slots."""
    n = x.shape[0]
    lo_core64, hi_core64 = args64["lo_core"], args64["hi_core"]
    b_out64 = args64["b_out"]
    numer = np.zeros(n, np.float64)
    denom = np.zeros(n, np.float64)
    sigm = lambda v: 1.0 / (1.0 + np.exp(-v))
    for s_ in range(S):
        idx = pts[s_]
        cnt = len(idx)
        if cnt == 0:
            continue
        xs = x[idx].astype(np.float64)
        arg_l = swin[s_] * (xs - lo_core64[s_])
        arg_r = swin[s_] * (hi_core64[s_] - xs)
        w = np.prod(sigm(arg_l) * sigm(arg_r), axis=-1)
        out_s = (o_by_sub(s_)[:cnt].astype(np.float64)
                 + b_out64[s_, 0]) * SCALE + SHIFT
        np.add.at(numer, idx, out_s * w)
        np.add.at(denom, idx, w)
    u = numer / (denom + 1e-8)
    x64 = x.astype(np.float64)
    gg = -np.sin(np.pi * x64[:, 1])
    fac = np.tanh(x64[:, 1] + 1.0) * np.tanh(x64[:, 1] - 1.0) * np.tanh(x64[:, 0])
    return (gg + fac * u)[:, None].astype(np.float32)


def kernel(x, lo_core, hi_core, lo_ext, hi_ext,
           W_in, b_in, W_h1, b_h1, W_h2, b_h2, W_out, b_out,
           _profile=False):
    x = np.asarray(x, np.float32)
    args64 = {k: np.asarray(v, np.float64) for k, v in dict(
        lo_core=lo_core, hi_core=hi_core, lo_ext=lo_ext, hi_ext=hi_ext,
        W_in=W_in, b_in=b_in, W_h1=W_h1, b_h1=b_h1, W_h2=W_h2, b_h2=b_h2,
        W_out=W_out, b_out=b_out).items()}

    prep = _prepare(x, args64)
    if prep is None:
        return _host_reference(x, **args64)
    pts, pad, swin, Wn, bn = prep

    in_maps = _pack(x, args64, pts, pad, Wn, bn)

    from concourse.bass_utils import run_bass_kernel_spmd
    if pad not in _BUILD_CACHE:
        _BUILD_CACHE[pad] = _build_bass(pad)
    nc = _BUILD_CACHE[pad]
    res = run_bass_kernel_spmd(nc, in_maps, list(range(NCORES)),
                               trace=bool(_profile))

    def o_by_sub(s_):
        c, rem = divmod(s_, SUB_PER_CORE)
        g, j = divmod(rem, 4)
        return res.results[c]["o"][g, j]

    final = _epilogue(x, args64, pts, swin, o_by_sub)
    if _profile:
        return final, res
    return final
